# revision 1
# baseline (speedup 1.0000x reference)
"""Trainium2 Bass kernel: masked-mean-pool -> linear projection -> pairwise L2.

Full computation:
    pooled = einsum('nlh,nl->nh', inputs, masks) / sum(masks, 1)   # [N, H]
    emb    = pooled @ W + b                                         # [N, H]
    out    = pairwise_l2(emb)                                       # [N, N]

Sharding: rows (N) split across 8 NeuronCores; each core pools/projects its
512-row shard, all-gathers an augmented embedding payload [-2*embT; sqnorm_row]
([513, 512] f32 per rank), and computes its [512, 4096] block of the distance
matrix with a single augmented matmul:
    psum[i, j] = sum_h embT[h,i] * (-2*embT[h,j]) + 1 * sn[j]  (K = 512 + 1)
    dist[i, j] = sqrt(max(psum[i,j] + sn[i], 0))
Host concatenates the 8 row-blocks and zeroes the diagonal.
"""

import sys
import numpy as np

if "/opt/trn_rl_repo" not in sys.path:
    sys.path.insert(0, "/opt/trn_rl_repo")

N_TOTAL, L, H = 4096, 64, 512
R = 8                    # cores
NS = N_TOTAL // R        # 512 rows per core
NB = NS // 128           # 4 n-blocks of 128 partitions
HT = H // 128            # 4 h-tiles of 128
LC = 4                   # l-chunks per n-block
LCS = L // LC            # 16 l per chunk
AUG = H + 1              # payload rows: 512 emb + 1 sq-norm

_CACHE = {}


def _build_nc(use_masks: bool, rep: int = 1, rep_scope: str = "all",
              skip_ag: bool = False, ag_mode: str = "normal",
              p1_mode: str = "tree", lcs: int = LCS, xbufs: int = 4):
    import concourse.bacc as bacc
    import concourse.tile as tile
    import concourse.mybir as mybir

    f32 = mybir.dt.float32
    ALU = mybir.AluOpType
    ACT = mybir.ActivationFunctionType

    nc = bacc.Bacc(
        "TRN2",
        target_bir_lowering=False,
        debug=False,
        enable_asserts=False,
        num_devices=R,
    )

    x_ext = nc.dram_tensor("inputs", [NS, L, H], f32, kind="ExternalInput")
    if use_masks:
        mw_ext = nc.dram_tensor("mw", [NS, L], f32, kind="ExternalInput")
    w_ext = nc.dram_tensor("W", [H, H], f32, kind="ExternalInput")
    b_ext = nc.dram_tensor("b", [H], f32, kind="ExternalInput")
    out_ext = nc.dram_tensor("out", [NS, N_TOTAL], f32, kind="ExternalOutput")

    ident_dram = nc.inline_tensor(np.eye(128, dtype=np.float32), name="ident")

    with tile.TileContext(nc) as tc:
        with (
            tc.tile_pool(name="const", bufs=1) as cpool,
            tc.tile_pool(name="xp", bufs=xbufs) as xpool,
            tc.tile_pool(name="rp", bufs=2) as rpool,
            tc.tile_pool(name="ep", bufs=4) as epool,
            tc.tile_pool(name="dram", bufs=1, space="DRAM") as dpool,
        ):
            # ---- constants / weights ----
            ident_sb = cpool.tile([128, 128], f32, name="ident_sb")
            nc.sync.dma_start(ident_sb[:, :], ident_dram[:, :])

            w_sb = cpool.tile([128, HT, H], f32, name="w_sb")
            for k in range(HT):
                nc.sync.dma_start(w_sb[:, k, :], w_ext[k * 128:(k + 1) * 128, :])

            b_ap = b_ext.ap().rearrange("(x y) -> x y", y=1)  # [512, 1]
            b_sb = cpool.tile([128, HT], f32, name="b_sb")
            for m in range(HT):
                nc.sync.dma_start(b_sb[:, m:m + 1], b_ap[m * 128:(m + 1) * 128, 0:1])
            b2_sb = cpool.tile([128, HT], f32, name="b2_sb")
            nc.vector.tensor_scalar_mul(b2_sb[:, :], b_sb[:, :], -2.0)

            ones_col = cpool.tile([128, 1], f32, name="ones_col")
            nc.vector.memset(ones_col[:, :], 1.0)
            ones_row = cpool.tile([1, 128], f32, name="ones_row")
            nc.vector.memset(ones_row[:, :], 1.0)

            if use_masks:
                mw_sb = cpool.tile([128, NB, L], f32, name="mw_sb")
                for nb in range(NB):
                    nc.sync.dma_start(
                        mw_sb[:, nb, :], mw_ext[nb * 128:(nb + 1) * 128, :]
                    )

            rep_p1 = rep if rep_scope in ("all", "p1") else 1
            rep_p23 = rep if rep_scope in ("all", "p23") else 1
            n_outer = rep if rep_scope == "all" else 1
            for _rep in range(n_outer):
                for _rp1 in range(rep_p1 if rep_scope == "p1" else 1):
                    # ---- phase 1: masked-mean pooling (n on partitions) ----
                    # Per chunk: GpSimd does tree level 1 (16->8), DVE does the
                    # rest (8->1); PE transpose-accumulates the chunk partial
                    # directly into pooledT PSUM banks (h on partitions).
                    pooledT_sb = cpool.tile([128, HT, NS], f32, name="pooledT_sb")
                    with tc.tile_pool(name="pstT", bufs=1, space="PSUM") as tpool:
                        psT = [
                            tpool.tile([128, NS], f32, name=f"psT{ht}")
                            for ht in range(HT)
                        ]
                        n_lc = L // lcs
                        for nb in range(NB):
                            for lc in range(n_lc):
                                xt = xpool.tile([128, lcs, H], f32, name="xt")
                                nc.sync.dma_start(
                                    xt[:, :, :],
                                    x_ext[nb * 128:(nb + 1) * 128,
                                          lc * lcs:(lc + 1) * lcs, :],
                                )
                                if use_masks:
                                    for l in range(lcs):
                                        gl = lc * lcs + l
                                        nc.scalar.mul(
                                            xt[:, l, :], xt[:, l, :],
                                            mw_sb[:, nb, gl:gl + 1]
                                        )
                                if p1_mode == "reduce":
                                    # single strided reduce over l per chunk
                                    part = xpool.tile([128, H], f32,
                                                      name="part", bufs=4)
                                    nc.vector.tensor_reduce(
                                        part[:, :],
                                        xt.rearrange("p l h -> p h l"),
                                        mybir.AxisListType.X, ALU.add,
                                    )
                                    psrc = part
                                    pidx = None
                                else:
                                    # binary-tree sum over l on DVE
                                    half = lcs
                                    while half > 1:
                                        half //= 2
                                        nc.vector.tensor_add(
                                            xt[:, 0:half, :], xt[:, 0:half, :],
                                            xt[:, half:2 * half, :]
                                        )
                                    psrc = xt
                                    pidx = 0
                                # PE: transpose-accumulate chunk partial into PSUM
                                for ht in range(HT):
                                    sl = (psrc[:, ht * 128:(ht + 1) * 128]
                                          if pidx is None else
                                          psrc[:, pidx, ht * 128:(ht + 1) * 128])
                                    nc.tensor.matmul(
                                        psT[ht][:, nb * 128:(nb + 1) * 128],
                                        sl,
                                        ident_sb[:, :],
                                        is_transpose=True,
                                        start=(lc == 0),
                                        stop=(lc == n_lc - 1),
                                    )
                        for ht in range(HT):
                            nc.vector.tensor_copy(pooledT_sb[:, ht, :], psT[ht][:, :])

                for _rp23 in range(rep_p23 if rep_scope == "p23" else 1):
                    # ---- phase 2b: projection embT = W.T-contract(pooledT) + b ----
                    embT_sb = cpool.tile([128, HT, NS], f32, name="embT_sb")
                    scaledT_sb = cpool.tile([128, HT, NS], f32, name="scaledT_sb")
                    with tc.tile_pool(name="psp", bufs=2, space="PSUM") as ppool:
                        for m in range(HT):
                            psp = ppool.tile([128, NS], f32, name="psp")
                            for k in range(HT):
                                nc.tensor.matmul(
                                    psp[:, :],
                                    w_sb[:, k, m * 128:(m + 1) * 128],
                                    pooledT_sb[:, k, :],
                                    start=(k == 0),
                                    stop=(k == HT - 1),
                                )
                            nc.scalar.activation(
                                embT_sb[:, m, :], psp[:, :], ACT.Identity,
                                bias=b_sb[:, m:m + 1], scale=1.0,
                            )
                            nc.scalar.activation(
                                scaledT_sb[:, m, :], psp[:, :], ACT.Identity,
                                bias=b2_sb[:, m:m + 1], scale=-2.0,
                            )

                    # ---- phase 2c: squared norms ----
                    sq_sb = cpool.tile([128, HT, NS], f32, name="sq_sb")
                    for k in range(HT):
                        nc.scalar.square(sq_sb[:, k, :], embT_sb[:, k, :])

                    sn_row_sb = cpool.tile([1, NS], f32, name="sn_row_sb")
                    sn_col_sb = cpool.tile([128, HT], f32, name="sn_col_sb")
                    with tc.tile_pool(name="psn", bufs=1, space="PSUM") as npool:
                        ps_snrow = npool.tile([1, NS], f32, name="ps_snrow")
                        for k in range(HT):
                            nc.tensor.matmul(
                                ps_snrow[0:1, :], ones_col[:, 0:1], sq_sb[:, k, :],
                                start=(k == 0), stop=(k == HT - 1),
                            )
                        nc.scalar.copy(sn_row_sb[0:1, :], ps_snrow[0:1, :])

                        for m in range(HT):
                            ps_sncol = npool.tile([128, 1], f32, name="ps_sncol", bufs=2)
                            for k in range(HT):
                                nc.tensor.matmul(
                                    ps_sncol[:, 0:1],
                                    sq_sb[:, k, m * 128:(m + 1) * 128],
                                    ones_col[:, 0:1],
                                    start=(k == 0),
                                    stop=(k == HT - 1),
                                )
                            nc.scalar.copy(sn_col_sb[:, m:m + 1], ps_sncol[:, 0:1])

                    # ---- phase 2d: all-gather payload [-2*embT ; sn_row] ----
                    payload_d = dpool.tile([AUG, NS], f32, name="payload_d")
                    gathered_d = dpool.tile(
                        [R * AUG, NS], f32, name="gathered_d", addr_space="Shared"
                    )
                    for k in range(HT):
                        nc.sync.dma_start(
                            payload_d[k * 128:(k + 1) * 128, :], scaledT_sb[:, k, :]
                        )
                    nc.sync.dma_start(payload_d[H:H + 1, :], sn_row_sb[0:1, :])
                    if ag_mode == "small":
                        # timing probe: tiny AG (one 128-row slice), phase 3
                        # reads local payload
                        gsmall_d = dpool.tile([R * 128, NS], f32,
                                              name="gsmall_d",
                                              addr_space="Shared")
                        nc.gpsimd.collective_compute(
                            "AllGather", ALU.bypass,
                            replica_groups=[list(range(R))],
                            ins=[payload_d[0:128, :].opt()],
                            outs=[gsmall_d.opt()],
                        )
                    elif ag_mode == "double":
                        # timing probe: two independent full-size AGs
                        # back-to-back
                        gat2_d = dpool.tile([R * AUG, NS], f32, name="gat2_d",
                                            addr_space="Shared")
                        nc.gpsimd.collective_compute(
                            "AllGather", ALU.bypass,
                            replica_groups=[list(range(R))],
                            ins=[payload_d.opt()],
                            outs=[gathered_d.opt()],
                        )
                        nc.gpsimd.collective_compute(
                            "AllGather", ALU.bypass,
                            replica_groups=[list(range(R))],
                            ins=[payload_d.opt()],
                            outs=[gat2_d.opt()],
                        )
                    elif not skip_ag:
                        nc.gpsimd.collective_compute(
                            "AllGather",
                            ALU.bypass,
                            replica_groups=[list(range(R))],
                            ins=[payload_d.opt()],
                            outs=[gathered_d.opt()],
                        )

                    # ---- phase 3: distance blocks ----
                    bpool_cm = tc.tile_pool(name="psb", bufs=4, space="PSUM")
                    bpool = bpool_cm.__enter__()
                    use_local = skip_ag or ag_mode in ("small",)
                    src_d = payload_d if use_local else gathered_d
                    for jb in range(R):
                        rhst = rpool.tile([128, HT, NS], f32, name="rhst")
                        snr = rpool.tile([1, NS], f32, name="snr")
                        base = 0 if use_local else jb * AUG
                        for k in range(HT):
                            nc.sync.dma_start(
                                rhst[:, k, :],
                                src_d[base + k * 128:base + (k + 1) * 128, :],
                            )
                        nc.sync.dma_start(snr[0:1, :], src_d[base + H:base + H + 1, :])
                        for m in range(HT):
                            ps = bpool.tile([128, NS], f32, name="ps")
                            nc.tensor.matmul(
                                ps[:, :], ones_row[0:1, :], snr[0:1, :],
                                start=True, stop=False,
                            )
                            for k in range(HT):
                                nc.tensor.matmul(
                                    ps[:, :],
                                    embT_sb[:, k, m * 128:(m + 1) * 128],
                                    rhst[:, k, :],
                                    start=False,
                                    stop=(k == HT - 1),
                                )
                            sqt = epool.tile([128, NS], f32, name="sqt")
                            nc.vector.tensor_scalar(
                                sqt[:, :], ps[:, :], sn_col_sb[:, m:m + 1], 0.0,
                                op0=ALU.add, op1=ALU.max,
                            )
                            nc.scalar.sqrt(sqt[:, :], sqt[:, :])
                            nc.sync.dma_start(
                                out_ext[m * 128:(m + 1) * 128, jb * NS:(jb + 1) * NS],
                                sqt[:, :],
                            )
                    bpool_cm.__exit__(None, None, None)

    nc.compile()
    return nc


def _get_nc(use_masks: bool, rep: int = 1):
    key = (use_masks, rep)
    if key not in _CACHE:
        _CACHE[key] = _build_nc(use_masks, rep)
    return _CACHE[key]


def _run_device(x, mw, w_eff, b, trace=False, trace_cores=None):
    from concourse import bass_utils

    use_masks = mw is not None
    nc = _get_nc(use_masks)
    in_maps = []
    for r in range(R):
        m = {
            "inputs": np.ascontiguousarray(x[r * NS:(r + 1) * NS]),
            "W": w_eff,
            "b": b,
        }
        if use_masks:
            m["mw"] = np.ascontiguousarray(mw[r * NS:(r + 1) * NS])
        in_maps.append(m)
    res = bass_utils.run_bass_kernel_spmd(
        nc,
        in_maps,
        core_ids=list(range(R)),
        trace=trace,
        trace_cores=trace_cores,
    )
    out = np.concatenate([res.results[r]["out"] for r in range(R)], axis=0)
    np.fill_diagonal(out, 0.0)
    return out, res


def kernel(inputs, masks, W, b):
    inputs = np.ascontiguousarray(np.asarray(inputs, dtype=np.float32))
    masks = np.asarray(masks, dtype=np.float32)
    W = np.ascontiguousarray(np.asarray(W, dtype=np.float32))
    b = np.ascontiguousarray(np.asarray(b, dtype=np.float32))

    denom = masks.sum(axis=1, keepdims=True)
    row_uniform = bool(np.all(masks == masks[:, :1])) and bool(np.all(denom != 0))
    if row_uniform:
        # uniform per-row masks cancel: pooled = mean over L; fold 1/L into W
        w_eff = np.ascontiguousarray(W / np.float32(L))
        out, _ = _run_device(inputs, None, w_eff, b)
    else:
        mw = np.ascontiguousarray((masks / denom).astype(np.float32))
        out, _ = _run_device(inputs, mw, W, b)
    return out



# revision 6
# speedup vs baseline: 1.1692x; 1.1692x over previous
"""Trainium2 Bass kernel: masked-mean-pool -> linear projection -> pairwise L2.

Full computation:
    pooled = einsum('nlh,nl->nh', inputs, masks) / sum(masks, 1)   # [N, H]
    emb    = pooled @ W + b                                         # [N, H]
    out    = pairwise_l2(emb)                                       # [N, N]

Sharding: rows (N) split across 8 NeuronCores; each core pools/projects its
512-row shard, all-gathers a bf16 payload [-2*embT ; sqnorm_row] ([513, 512]
per rank), and computes its [512, 4096] block of the distance matrix:
    psum[i, j] = 1*sn[j] + sum_h embT[h,i] * (-2*embT[h,j])
    dist[i, j] = sqrt(max(psum[i,j] + sn[i], 0))
Host concatenates the 8 row-blocks and zeroes the diagonal.

Perf structure (cost-model driven):
  - phase 1 pooling is HBM-bound (64 MB/core): input chunks stream on both
    HWDGE queues (sync + scalar) while the DVE tree-reduces over L and PE
    transposes pooled [n,h] chunks into [h,n].
  - the shard is processed in `split` column chunks: each chunk is projected
    and its payload all-gathered while later chunks are still streaming, so
    only the last chunk's (1/split-sized) all-gather is exposed — and even
    that overlaps the first chunks' distance matmuls.
  - payload writes + collectives ride the gpsimd (SWDGE) queue so a stalled
    payload write can never head-of-line-block the streaming queues.
  - phases 2/3 run all matmuls in bf16 (1 cycle/row vs 4 for fp32), which
    also halves the all-gather payload and the gathered read-back.
"""

import sys
import numpy as np

if "/opt/trn_rl_repo" not in sys.path:
    sys.path.insert(0, "/opt/trn_rl_repo")

N_TOTAL, L, H = 4096, 64, 512
R = 8                    # cores
NS = N_TOTAL // R        # 512 rows per core
NB = NS // 128           # 4 n-blocks of 128 partitions
HT = H // 128            # 4 h-tiles of 128
LC = 4                   # l-chunks per n-block (tree mode)
LCS = L // LC            # 16 l per chunk
AUG = H + 1              # payload rows: 512 emb + 1 sq-norm

_CACHE = {}


def _build_nc(use_masks: bool, rep: int = 1, rep_scope: str = "all",
              skip_ag: bool = False, p1_mode: str = "tree", split: int = 2):
    import concourse.bacc as bacc
    import concourse.tile as tile
    import concourse.mybir as mybir

    f32 = mybir.dt.float32
    bf16 = mybir.dt.bfloat16
    ALU = mybir.AluOpType
    ACT = mybir.ActivationFunctionType

    assert NB % split == 0
    NBC = NB // split        # n-blocks (column blocks of 128) per chunk
    CW = NS // split         # columns per chunk

    nc = bacc.Bacc(
        "TRN2",
        target_bir_lowering=False,
        debug=False,
        enable_asserts=False,
        num_devices=R,
    )

    x_ext = nc.dram_tensor("inputs", [NS, L, H], f32, kind="ExternalInput")
    if use_masks:
        mw_ext = nc.dram_tensor("mw", [NS, L], f32, kind="ExternalInput")
    w_ext = nc.dram_tensor("W", [H, H], f32, kind="ExternalInput")
    b_ext = nc.dram_tensor("b", [H], f32, kind="ExternalInput")
    out_ext = nc.dram_tensor("out", [NS, N_TOTAL], f32, kind="ExternalOutput")

    ident_dram = nc.inline_tensor(np.eye(128, dtype=np.float32), name="ident")

    with tile.TileContext(nc) as tc:
        with (
            tc.tile_pool(name="const", bufs=1) as cpool,
            tc.tile_pool(name="xp", bufs=4) as xpool,
            tc.tile_pool(name="rp", bufs=2) as rpool,
            tc.tile_pool(name="ep", bufs=4) as epool,
            tc.tile_pool(name="dram", bufs=1, space="DRAM") as dpool,
        ):
            # ---- constants / weights ----
            ident_sb = cpool.tile([128, 128], f32, name="ident_sb")
            nc.sync.dma_start(ident_sb[:, :], ident_dram[:, :])

            w_sb = cpool.tile([128, HT, H], f32, name="w_sb")
            for k in range(HT):
                nc.sync.dma_start(w_sb[:, k, :], w_ext[k * 128:(k + 1) * 128, :])
            w_bf = cpool.tile([128, HT, H], bf16, name="w_bf")
            nc.vector.tensor_copy(w_bf[:, :, :], w_sb[:, :, :])

            b_ap = b_ext.ap().rearrange("(x y) -> x y", y=1)  # [512, 1]
            b_sb = cpool.tile([128, HT], f32, name="b_sb")
            for m in range(HT):
                nc.sync.dma_start(b_sb[:, m:m + 1], b_ap[m * 128:(m + 1) * 128, 0:1])
            b2_sb = cpool.tile([128, HT], f32, name="b2_sb")
            nc.vector.tensor_scalar_mul(b2_sb[:, :], b_sb[:, :], -2.0)

            ones_col = cpool.tile([128, 1], bf16, name="ones_col")
            nc.vector.memset(ones_col[:, :], 1.0)
            ones_row = cpool.tile([1, 128], bf16, name="ones_row")
            nc.vector.memset(ones_row[:, :], 1.0)

            if use_masks:
                mw_sb = cpool.tile([128, NB, L], f32, name="mw_sb")
                for nb in range(NB):
                    nc.sync.dma_start(
                        mw_sb[:, nb, :], mw_ext[nb * 128:(nb + 1) * 128, :]
                    )

            rep_p1 = rep if rep_scope == "p1" else 1
            rep_p23 = rep if rep_scope == "p23" else 1
            n_outer = rep if rep_scope == "all" else 1

            def phase1_chunk(c, pooledT_bf, tpool):
                for nbl in range(NBC):
                    nb = c * NBC + nbl
                    part_sum = xpool.tile([128, H], f32, name="xsum", bufs=2)
                    for lc in range(LC):
                        xt = xpool.tile([128, LCS, H], f32, name="xt")
                        qq = nc.sync if (nb * LC + lc) % 2 == 0 else nc.scalar
                        qq.dma_start(
                            xt[:, :, :],
                            x_ext[nb * 128:(nb + 1) * 128,
                                  lc * LCS:(lc + 1) * LCS, :],
                        )
                        if use_masks:
                            for l in range(LCS):
                                gl = lc * LCS + l
                                nc.scalar.mul(
                                    xt[:, l, :], xt[:, l, :],
                                    mw_sb[:, nb, gl:gl + 1]
                                )
                        half = LCS
                        while half > 1:
                            half //= 2
                            nc.vector.tensor_add(
                                xt[:, 0:half, :], xt[:, 0:half, :],
                                xt[:, half:2 * half, :]
                            )
                        if lc == 0:
                            nc.vector.tensor_copy(part_sum[:, :], xt[:, 0, :])
                        else:
                            nc.vector.tensor_add(
                                part_sum[:, :], part_sum[:, :], xt[:, 0, :])
                    # PE: transpose pooled chunk into [h, n] layout
                    pst = tpool.tile([128, HT, 128], f32, name="pst")
                    for ht in range(HT):
                        nc.tensor.matmul(
                            pst[:, ht, :],
                            part_sum[:, ht * 128:(ht + 1) * 128],
                            ident_sb[:, :],
                            is_transpose=True,
                            start=True, stop=True,
                        )
                    nc.vector.tensor_copy(
                        pooledT_bf[:, :, nb * 128:(nb + 1) * 128],
                        pst[:, :, :])

            def phase2_chunk(c, pooledT_bf, embT_bf, scaledT_bf, sq_bf,
                             snrow_bf, sn_col_sb, payload, ppool, npool):
                cs = c * CW
                for m in range(HT):
                    psp = ppool.tile([128, CW], f32, name="psp")
                    for k in range(HT):
                        nc.tensor.matmul(
                            psp[:, :],
                            w_bf[:, k, m * 128:(m + 1) * 128],
                            pooledT_bf[:, k, cs:cs + CW],
                            start=(k == 0),
                            stop=(k == HT - 1),
                        )
                    nc.scalar.activation(
                        scaledT_bf[:, m, cs:cs + CW], psp[:, :], ACT.Identity,
                        bias=b2_sb[:, m:m + 1], scale=-2.0,
                    )
                    nc.gpsimd.dma_start(
                        payload[m * 128:(m + 1) * 128, :],
                        scaledT_bf[:, m, cs:cs + CW])
                    nc.scalar.activation(
                        embT_bf[:, m, cs:cs + CW], psp[:, :], ACT.Identity,
                        bias=b_sb[:, m:m + 1], scale=1.0,
                    )
                    nc.scalar.square(sq_bf[:, m, cs:cs + CW],
                                     embT_bf[:, m, cs:cs + CW])

                # squared norms: row vector for this chunk's columns
                ps_snrow = npool.tile([1, CW], f32, name="ps_snrow")
                for k in range(HT):
                    nc.tensor.matmul(
                        ps_snrow[0:1, :], ones_col[:, 0:1],
                        sq_bf[:, k, cs:cs + CW],
                        start=(k == 0), stop=(k == HT - 1),
                    )
                nc.scalar.copy(snrow_bf[0:1, cs:cs + CW], ps_snrow[0:1, :])
                nc.gpsimd.dma_start(payload[H:H + 1, :],
                                    snrow_bf[0:1, cs:cs + CW])

                # per-local-row norms for this chunk's column blocks
                for mcl in range(NBC):
                    mc = c * NBC + mcl
                    ps_sncol = npool.tile([128, 1], f32, name="ps_sncol")
                    for k in range(HT):
                        nc.tensor.matmul(
                            ps_sncol[:, 0:1],
                            sq_bf[:, k, mc * 128:(mc + 1) * 128],
                            ones_col[:, 0:1],
                            start=(k == 0),
                            stop=(k == HT - 1),
                        )
                    nc.scalar.copy(sn_col_sb[:, mc:mc + 1], ps_sncol[:, 0:1])

            def phase3_chunk(c, embT_bf, sn_col_sb, src_d, bpool, local):
                for jb in range(R):
                    rhst = rpool.tile([128, HT, CW], bf16, name="rhst")
                    snr = rpool.tile([1, CW], bf16, name="snr")
                    base = 0 if local else jb * AUG
                    for k in range(HT):
                        nc.scalar.dma_start(
                            rhst[:, k, :],
                            src_d[base + k * 128:base + (k + 1) * 128, :],
                        )
                    nc.scalar.dma_start(
                        snr[0:1, :], src_d[base + H:base + H + 1, :])
                    for m in range(HT):
                        ps = bpool.tile([128, CW], f32, name="ps")
                        nc.tensor.matmul(
                            ps[:, :], ones_row[0:1, :], snr[0:1, :],
                            start=True, stop=False,
                        )
                        for k in range(HT):
                            nc.tensor.matmul(
                                ps[:, :],
                                embT_bf[:, k, m * 128:(m + 1) * 128],
                                rhst[:, k, :],
                                start=False,
                                stop=(k == HT - 1),
                            )
                        sqt = epool.tile([128, CW], f32, name="sqt")
                        nc.vector.tensor_scalar(
                            sqt[:, :], ps[:, :], sn_col_sb[:, m:m + 1],
                            0.0, op0=ALU.add, op1=ALU.max,
                        )
                        nc.scalar.sqrt(sqt[:, :], sqt[:, :])
                        nc.sync.dma_start(
                            out_ext[m * 128:(m + 1) * 128,
                                    jb * NS + c * CW:jb * NS + c * CW + CW],
                            sqt[:, :],
                        )

            for _rep in range(n_outer):
                pooledT_bf = cpool.tile([128, HT, NS], bf16, name="pooledT_bf")
                embT_bf = cpool.tile([128, HT, NS], bf16, name="embT_bf")
                scaledT_bf = cpool.tile([128, HT, NS], bf16, name="scaledT_bf")
                sq_bf = cpool.tile([128, HT, NS], bf16, name="sq_bf")
                snrow_bf = cpool.tile([1, NS], bf16, name="snrow_bf")
                sn_col_sb = cpool.tile([128, HT], f32, name="sn_col_sb")
                payloads = [
                    dpool.tile([AUG, CW], bf16, name=f"payload{c}_d")
                    for c in range(split)
                ]
                gathereds = [
                    dpool.tile([R * AUG, CW], bf16, name=f"gathered{c}_d",
                               addr_space="Shared")
                    for c in range(split)
                ]

                if rep_scope == "p1":
                    with tc.tile_pool(name="pstT", bufs=2, space="PSUM") as tpool:
                        for _ in range(rep_p1):
                            for c in range(split):
                                phase1_chunk(c, pooledT_bf, tpool)
                    # still produce phases 2/3 once so outputs exist
                ph1_done = rep_scope == "p1"

                for _rp23 in range(rep_p23):
                    first = _rp23 == 0
                    with (
                        tc.tile_pool(name="pstT", bufs=2, space="PSUM") as tpool,
                        tc.tile_pool(name="psp", bufs=2, space="PSUM") as ppool,
                        tc.tile_pool(name="psn", bufs=2, space="PSUM") as npool,
                    ):
                        for c in range(split):
                            if not ph1_done and (rep_scope != "p23" or first):
                                phase1_chunk(c, pooledT_bf, tpool)
                            phase2_chunk(c, pooledT_bf, embT_bf, scaledT_bf,
                                         sq_bf, snrow_bf, sn_col_sb,
                                         payloads[c], ppool, npool)
                            if not skip_ag:
                                nc.gpsimd.collective_compute(
                                    "AllGather",
                                    ALU.bypass,
                                    replica_groups=[list(range(R))],
                                    ins=[payloads[c].opt()],
                                    outs=[gathereds[c].opt()],
                                )
                    with tc.tile_pool(name="psb", bufs=4, space="PSUM") as bpool:
                        for c in range(split):
                            src = payloads[c] if skip_ag else gathereds[c]
                            phase3_chunk(c, embT_bf, sn_col_sb, src, bpool,
                                         skip_ag)

    nc.compile()
    return nc


def _get_nc(use_masks: bool, rep: int = 1):
    key = (use_masks, rep)
    if key not in _CACHE:
        _CACHE[key] = _build_nc(use_masks, rep)
    return _CACHE[key]


def _run_device(x, mw, w_eff, b, trace=False, trace_cores=None):
    from concourse import bass_utils

    use_masks = mw is not None
    nc = _get_nc(use_masks)
    in_maps = []
    for r in range(R):
        m = {
            "inputs": np.ascontiguousarray(x[r * NS:(r + 1) * NS]),
            "W": w_eff,
            "b": b,
        }
        if use_masks:
            m["mw"] = np.ascontiguousarray(mw[r * NS:(r + 1) * NS])
        in_maps.append(m)
    res = bass_utils.run_bass_kernel_spmd(
        nc,
        in_maps,
        core_ids=list(range(R)),
        trace=trace,
        trace_cores=trace_cores,
    )
    out = np.concatenate([res.results[r]["out"] for r in range(R)], axis=0)
    np.fill_diagonal(out, 0.0)
    return out, res


def kernel(inputs, masks, W, b):
    inputs = np.ascontiguousarray(np.asarray(inputs, dtype=np.float32))
    masks = np.asarray(masks, dtype=np.float32)
    W = np.ascontiguousarray(np.asarray(W, dtype=np.float32))
    b = np.ascontiguousarray(np.asarray(b, dtype=np.float32))

    denom = masks.sum(axis=1, keepdims=True)
    row_uniform = bool(np.all(masks == masks[:, :1])) and bool(np.all(denom != 0))
    if row_uniform:
        # uniform per-row masks cancel: pooled = mean over L; fold 1/L into W
        w_eff = np.ascontiguousarray(W / np.float32(L))
        out, _ = _run_device(inputs, None, w_eff, b)
    else:
        mw = np.ascontiguousarray((masks / denom).astype(np.float32))
        out, _ = _run_device(inputs, mw, W, b)
    return out


# revision 17
# speedup vs baseline: 1.3240x; 1.1324x over previous
"""Trainium2 Bass kernel: masked-mean-pool -> linear projection -> pairwise L2.

Full computation:
    pooled = einsum('nlh,nl->nh', inputs, masks) / sum(masks, 1)   # [N, H]
    emb    = pooled @ W + b                                         # [N, H]
    out    = pairwise_l2(emb)                                       # [N, N]

Sharding: rows (N) split across 8 NeuronCores; each core pools/projects its
512-row shard, all-gathers a bf16 payload [-2*embT ; sqnorm_row] ([513, 512]
per rank), and computes its [512, 4096] block of the distance matrix:
    psum[i, j] = 1*sn[j] + sum_h embT[h,i] * (-2*embT[h,j])
    dist[i, j] = sqrt(max(psum[i,j] + sn[i], 0))
Host concatenates the 8 row-blocks and zeroes the diagonal.

Perf structure (cost-model driven):
  - phase 1 pooling is HBM-bound (64 MB/core): input chunks stream on both
    HWDGE queues (sync + scalar) while the DVE tree-reduces over L and PE
    transposes pooled [n,h] chunks into [h,n].
  - the shard is processed in `split` column chunks: each chunk is projected
    and its payload all-gathered while later chunks are still streaming, so
    only the last chunk's (1/split-sized) all-gather is exposed — and even
    that overlaps the first chunks' distance matmuls.
  - payload writes + collectives ride the gpsimd (SWDGE) queue so a stalled
    payload write can never head-of-line-block the streaming queues.
  - phases 2/3 run all matmuls in bf16 (1 cycle/row vs 4 for fp32), which
    also halves the all-gather payload and the gathered read-back.
"""

import sys
import numpy as np

if "/opt/trn_rl_repo" not in sys.path:
    sys.path.insert(0, "/opt/trn_rl_repo")

N_TOTAL, L, H = 4096, 64, 512
R = 8                    # cores
NS = N_TOTAL // R        # 512 rows per core
NB = NS // 128           # 4 n-blocks of 128 partitions
HT = H // 128            # 4 h-tiles of 128
LC = 4                   # l-chunks per n-block (tree mode)
LCS = L // LC            # 16 l per chunk
AUG = H + 1              # payload rows: 512 emb + 1 sq-norm

_CACHE = {}


def _build_nc(use_masks: bool, rep: int = 1, rep_scope: str = "all",
              skip_ag: bool = False, p1_mode: str = "tree", split: int = 1,
              warm_n: int = 40, ag_flat: bool = True, out_bf: bool = True,
              tree_bf: bool = True, tree_gp: bool = True):
    import concourse.bacc as bacc
    import concourse.tile as tile
    import concourse.mybir as mybir

    f32 = mybir.dt.float32
    bf16 = mybir.dt.bfloat16
    ALU = mybir.AluOpType
    ACT = mybir.ActivationFunctionType

    assert NB % split == 0
    NBC = NB // split        # n-blocks (column blocks of 128) per chunk
    CW = NS // split         # columns per chunk

    nc = bacc.Bacc(
        "TRN2",
        target_bir_lowering=False,
        debug=False,
        enable_asserts=False,
        num_devices=R,
    )

    x_ext = nc.dram_tensor("inputs", [NS, L, H], f32, kind="ExternalInput")
    if use_masks:
        mw_ext = nc.dram_tensor("mw", [NS, L], f32, kind="ExternalInput")
    w_ext = nc.dram_tensor("W", [H, H], f32, kind="ExternalInput")
    b_ext = nc.dram_tensor("b", [H], f32, kind="ExternalInput")
    out_dt = bf16 if out_bf else f32
    out_ext = nc.dram_tensor("out", [NS, N_TOTAL], out_dt, kind="ExternalOutput")

    ident_dram = nc.inline_tensor(np.eye(128, dtype=np.float32), name="ident")

    with tile.TileContext(nc) as tc:
        with (
            tc.tile_pool(name="const", bufs=1) as cpool,
            tc.tile_pool(name="xp", bufs=4) as xpool,
            tc.tile_pool(name="rp", bufs=2) as rpool,
            tc.tile_pool(name="ep", bufs=4) as epool,
            tc.tile_pool(name="dram", bufs=1, space="DRAM") as dpool,
        ):
            # ---- constants / weights ----
            ident_sb = cpool.tile([128, 128], f32, name="ident_sb")
            nc.sync.dma_start(ident_sb[:, :], ident_dram[:, :])

            w_sb = cpool.tile([128, HT, H], f32, name="w_sb")
            for k in range(HT):
                nc.sync.dma_start(w_sb[:, k, :], w_ext[k * 128:(k + 1) * 128, :])
            w_bf = cpool.tile([128, HT, H], bf16, name="w_bf")
            nc.vector.tensor_copy(w_bf[:, :, :], w_sb[:, :, :])

            b_ap = b_ext.ap().rearrange("(x y) -> x y", y=1)  # [512, 1]
            b_sb = cpool.tile([128, HT], f32, name="b_sb")
            for m in range(HT):
                nc.sync.dma_start(b_sb[:, m:m + 1], b_ap[m * 128:(m + 1) * 128, 0:1])
            b2_sb = cpool.tile([128, HT], f32, name="b2_sb")
            nc.vector.tensor_scalar_mul(b2_sb[:, :], b_sb[:, :], -2.0)

            ones_col = cpool.tile([128, 1], bf16, name="ones_col")
            nc.vector.memset(ones_col[:, :], 1.0)
            ones_row = cpool.tile([1, 128], bf16, name="ones_row")
            nc.vector.memset(ones_row[:, :], 1.0)

            if use_masks:
                mw_sb = cpool.tile([128, NB, L], f32, name="mw_sb")
                for nb in range(NB):
                    nc.sync.dma_start(
                        mw_sb[:, nb, :], mw_ext[nb * 128:(nb + 1) * 128, :]
                    )

            rep_p1 = rep if rep_scope == "p1" else 1
            rep_p23 = rep if rep_scope == "p23" else 1
            n_outer = rep if rep_scope == "all" else 1

            def phase1_chunk(c, pooledT_bf, tpool):
                for nbl in range(NBC):
                    nb = c * NBC + nbl
                    part_sum = xpool.tile([128, H], f32, name="xsum", bufs=2)
                    for lc in range(LC):
                        xt = xpool.tile([128, LCS, H], f32, name="xt")
                        qq = nc.sync if (nb * LC + lc) % 2 == 0 else nc.scalar
                        qq.dma_start(
                            xt[:, :, :],
                            x_ext[nb * 128:(nb + 1) * 128,
                                  lc * LCS:(lc + 1) * LCS, :],
                        )
                        if use_masks:
                            for l in range(LCS):
                                gl = lc * LCS + l
                                nc.scalar.mul(
                                    xt[:, l, :], xt[:, l, :],
                                    mw_sb[:, nb, gl:gl + 1]
                                )
                        # binary-tree sum over l; chunks alternate between DVE
                        # and the otherwise-idle GpSimd engine, and the lower
                        # tree levels run in bf16 (2x element rate)
                        eng = nc.vector
                        if tree_gp and (nb * LC + lc) % 2 == 1:
                            eng = nc.gpsimd
                        half = LCS
                        if tree_bf:
                            half //= 2
                            xb = xpool.tile([128, LCS // 2, H], bf16,
                                            name="xb", bufs=2)
                            eng.tensor_add(
                                xb[:, :, :], xt[:, 0:half, :],
                                xt[:, half:2 * half, :])
                            src = xb
                        else:
                            src = xt
                        while half > 1:
                            half //= 2
                            eng.tensor_add(
                                src[:, 0:half, :], src[:, 0:half, :],
                                src[:, half:2 * half, :]
                            )
                        if lc == 0:
                            eng.tensor_copy(part_sum[:, :], src[:, 0, :])
                        else:
                            eng.tensor_add(
                                part_sum[:, :], part_sum[:, :], src[:, 0, :])
                    # PE: transpose pooled chunk into [h, n] layout
                    pst = tpool.tile([128, HT, 128], f32, name="pst")
                    for ht in range(HT):
                        nc.tensor.matmul(
                            pst[:, ht, :],
                            part_sum[:, ht * 128:(ht + 1) * 128],
                            ident_sb[:, :],
                            is_transpose=True,
                            start=True, stop=True,
                        )
                    nc.vector.tensor_copy(
                        pooledT_bf[:, :, nb * 128:(nb + 1) * 128],
                        pst[:, :, :])

            def phase2_chunk(c, pooledT_bf, embT_bf, scaledT_bf, sq_bf,
                             snrow_bf, sn_col_sb, payload, ppool, npool):
                cs = c * CW
                for m in range(HT):
                    psp = ppool.tile([128, CW], f32, name="psp")
                    for k in range(HT):
                        nc.tensor.matmul(
                            psp[:, :],
                            w_bf[:, k, m * 128:(m + 1) * 128],
                            pooledT_bf[:, k, cs:cs + CW],
                            start=(k == 0),
                            stop=(k == HT - 1),
                        )
                    nc.scalar.activation(
                        scaledT_bf[:, m, cs:cs + CW], psp[:, :], ACT.Identity,
                        bias=b2_sb[:, m:m + 1], scale=-2.0,
                    )
                    nc.sync.dma_start(
                        payload[m * 128:(m + 1) * 128, :],
                        scaledT_bf[:, m, cs:cs + CW])
                    nc.scalar.activation(
                        embT_bf[:, m, cs:cs + CW], psp[:, :], ACT.Identity,
                        bias=b_sb[:, m:m + 1], scale=1.0,
                    )
                    nc.scalar.square(sq_bf[:, m, cs:cs + CW],
                                     embT_bf[:, m, cs:cs + CW])

                # squared norms: row vector for this chunk's columns
                ps_snrow = npool.tile([1, CW], f32, name="ps_snrow")
                for k in range(HT):
                    nc.tensor.matmul(
                        ps_snrow[0:1, :], ones_col[:, 0:1],
                        sq_bf[:, k, cs:cs + CW],
                        start=(k == 0), stop=(k == HT - 1),
                    )
                nc.scalar.copy(snrow_bf[0:1, cs:cs + CW], ps_snrow[0:1, :])
                nc.sync.dma_start(payload[H:H + 1, :],
                                  snrow_bf[0:1, cs:cs + CW])

                # per-local-row norms for this chunk's column blocks
                for mcl in range(NBC):
                    mc = c * NBC + mcl
                    ps_sncol = npool.tile([128, 1], f32, name="ps_sncol")
                    for k in range(HT):
                        nc.tensor.matmul(
                            ps_sncol[:, 0:1],
                            sq_bf[:, k, mc * 128:(mc + 1) * 128],
                            ones_col[:, 0:1],
                            start=(k == 0),
                            stop=(k == HT - 1),
                        )
                    nc.scalar.copy(sn_col_sb[:, mc:mc + 1], ps_sncol[:, 0:1])

            def phase3_chunk(c, embT_bf, sn_col_sb, src_d, bpool, local):
                for jb in range(R):
                    rhst = rpool.tile([128, HT, CW], bf16, name="rhst")
                    snr = rpool.tile([1, CW], bf16, name="snr")
                    base = 0 if local else jb * AUG
                    for k in range(HT):
                        nc.scalar.dma_start(
                            rhst[:, k, :],
                            src_d[base + k * 128:base + (k + 1) * 128, :],
                        )
                    nc.scalar.dma_start(
                        snr[0:1, :], src_d[base + H:base + H + 1, :])
                    for m in range(HT):
                        ps = bpool.tile([128, CW], f32, name="ps")
                        nc.tensor.matmul(
                            ps[:, :], ones_row[0:1, :], snr[0:1, :],
                            start=True, stop=False,
                        )
                        for k in range(HT):
                            nc.tensor.matmul(
                                ps[:, :],
                                embT_bf[:, k, m * 128:(m + 1) * 128],
                                rhst[:, k, :],
                                start=False,
                                stop=(k == HT - 1),
                            )
                        sqt = epool.tile([128, CW], f32, name="sqt")
                        nc.vector.tensor_scalar(
                            sqt[:, :], ps[:, :], sn_col_sb[:, m:m + 1],
                            0.0, op0=ALU.add, op1=ALU.max,
                        )
                        sqo = epool.tile([128, CW], out_dt, name="sqo")
                        nc.scalar.sqrt(sqo[:, :], sqt[:, :])
                        nc.sync.dma_start(
                            out_ext[m * 128:(m + 1) * 128,
                                    jb * NS + c * CW:jb * NS + c * CW + CW],
                            sqo[:, :],
                        )

            for _rep in range(n_outer):
                pooledT_bf = cpool.tile([128, HT, NS], bf16, name="pooledT_bf")
                embT_bf = cpool.tile([128, HT, NS], bf16, name="embT_bf")
                scaledT_bf = cpool.tile([128, HT, NS], bf16, name="scaledT_bf")
                sq_bf = cpool.tile([128, HT, NS], bf16, name="sq_bf")
                snrow_bf = cpool.tile([1, NS], bf16, name="snrow_bf")
                sn_col_sb = cpool.tile([128, HT], f32, name="sn_col_sb")
                payloads = [
                    dpool.tile([AUG, CW], bf16, name=f"payload{c}_d")
                    for c in range(split)
                ]
                gathereds = [
                    dpool.tile([R * AUG, CW], bf16, name=f"gathered{c}_d",
                               addr_space="Shared")
                    for c in range(split)
                ]

                if rep_scope == "p1":
                    with tc.tile_pool(name="pstT", bufs=2, space="PSUM") as tpool:
                        for _ in range(rep_p1):
                            for c in range(split):
                                phase1_chunk(c, pooledT_bf, tpool)
                    # still produce phases 2/3 once so outputs exist
                ph1_done = rep_scope == "p1"

                for _rp23 in range(rep_p23):
                    first = _rp23 == 0
                    with (
                        tc.tile_pool(name="pstT", bufs=2, space="PSUM") as tpool,
                        tc.tile_pool(name="psp", bufs=2, space="PSUM") as ppool,
                        tc.tile_pool(name="psn", bufs=2, space="PSUM") as npool,
                    ):
                        for c in range(split):
                            if not ph1_done and (rep_scope != "p23" or first):
                                phase1_chunk(c, pooledT_bf, tpool)
                            phase2_chunk(c, pooledT_bf, embT_bf, scaledT_bf,
                                         sq_bf, snrow_bf, sn_col_sb,
                                         payloads[c], ppool, npool)
                            if not skip_ag:
                                if ag_flat:
                                    ag_in = payloads[c][:, :].flatten().opt()
                                    ag_out = gathereds[c][:, :].flatten().opt()
                                else:
                                    ag_in = payloads[c].opt()
                                    ag_out = gathereds[c].opt()
                                nc.gpsimd.collective_compute(
                                    "AllGather",
                                    ALU.bypass,
                                    replica_groups=[list(range(R))],
                                    ins=[ag_in],
                                    outs=[ag_out],
                                )
                            if c == split - 1 and warm_n > 0:
                                # keep the PE's HAM clock-gate open while the
                                # all-gather runs: discarded CW-row matmuls
                                wps = ppool.tile([128, CW], f32, name="psp")
                                for wi in range(warm_n):
                                    nc.tensor.matmul(
                                        wps[:, :],
                                        embT_bf[:, wi % HT, 0:128],
                                        scaledT_bf[:, wi % HT, 0:CW],
                                        start=True, stop=True,
                                        skip_group_check=True,
                                    )
                                wsink = epool.tile([1, 1], f32, name="wsink")
                                nc.vector.tensor_copy(
                                    wsink[0:1, 0:1], wps[0:1, 0:1])
                    with tc.tile_pool(name="psb", bufs=4, space="PSUM") as bpool:
                        for c in range(split):
                            src = payloads[c] if skip_ag else gathereds[c]
                            phase3_chunk(c, embT_bf, sn_col_sb, src, bpool,
                                         skip_ag)

    nc.compile()
    return nc


def _get_nc(use_masks: bool, rep: int = 1):
    key = (use_masks, rep)
    if key not in _CACHE:
        _CACHE[key] = _build_nc(use_masks, rep)
    return _CACHE[key]


def _run_device(x, mw, w_eff, b, trace=False, trace_cores=None):
    from concourse import bass_utils

    use_masks = mw is not None
    nc = _get_nc(use_masks)
    in_maps = []
    for r in range(R):
        m = {
            "inputs": np.ascontiguousarray(x[r * NS:(r + 1) * NS]),
            "W": w_eff,
            "b": b,
        }
        if use_masks:
            m["mw"] = np.ascontiguousarray(mw[r * NS:(r + 1) * NS])
        in_maps.append(m)
    res = bass_utils.run_bass_kernel_spmd(
        nc,
        in_maps,
        core_ids=list(range(R)),
        trace=trace,
        trace_cores=trace_cores,
    )
    out = np.concatenate(
        [np.asarray(res.results[r]["out"]).astype(np.float32) for r in range(R)],
        axis=0,
    )
    np.fill_diagonal(out, 0.0)
    return out, res


def kernel(inputs, masks, W, b):
    inputs = np.ascontiguousarray(np.asarray(inputs, dtype=np.float32))
    masks = np.asarray(masks, dtype=np.float32)
    W = np.ascontiguousarray(np.asarray(W, dtype=np.float32))
    b = np.ascontiguousarray(np.asarray(b, dtype=np.float32))

    denom = masks.sum(axis=1, keepdims=True)
    row_uniform = bool(np.all(masks == masks[:, :1])) and bool(np.all(denom != 0))
    if row_uniform:
        # uniform per-row masks cancel: pooled = mean over L; fold 1/L into W
        w_eff = np.ascontiguousarray(W / np.float32(L))
        out, _ = _run_device(inputs, None, w_eff, b)
    else:
        mw = np.ascontiguousarray((masks / denom).astype(np.float32))
        out, _ = _run_device(inputs, mw, W, b)
    return out


# revision 29
# speedup vs baseline: 1.5791x; 1.1927x over previous
"""Trainium2 Bass kernel: masked-mean-pool -> linear projection -> pairwise L2.

Full computation:
    pooled = einsum('nlh,nl->nh', inputs, masks) / sum(masks, 1)   # [N, H]
    emb    = pooled @ W + b                                         # [N, H]
    out    = pairwise_l2(emb)                                       # [N, N]

Sharding: rows (N) split across 8 NeuronCores; each core pools/projects its
512-row shard, all-gathers a bf16 payload [-2*embT ; sqnorm_row] ([513, 512]
per rank), and computes its [512, 4096] block of the distance matrix:
    psum[i, j] = 1*sn[j] + sum_h embT[h,i] * (-2*embT[h,j])
    dist[i, j] = sqrt(max(psum[i,j] + sn[i], 0))
Host concatenates the 8 row-blocks and zeroes the diagonal.

Perf structure (HW-measured on trn2):
  - phase 1 pooling is HBM-bound (64 MB/core, ~180 us floor at 358 GB/s):
    8 MB input chunks alternate across both HWDGE queues (sync + scalar);
    the DVE tree-reduces each chunk over L (lower levels in bf16 for 2x
    element rate) and the PE transpose-accumulates the per-chunk partials
    into 4 persistent PSUM banks, keeping chunks fully independent.
  - phases 2/3 run all matmuls in bf16 (1 cycle/row vs 4 for fp32), which
    also halves the all-gather payload, the gathered read-back, and (with
    a bf16 output) the distance-matrix write-back.
  - the all-gather is a single collective with flattened 1-D APs (measured
    ~25-30 us, fixed-cost dominated; collectives act as full sync points on
    this runtime, so one big AG beats any split/overlap scheme).

Measured probes informing this layout (rep-9 dispatch-slope timing):
  splitting the AG 2/4-ways costs +25 us per extra collective; fp32
  matmuls in phase 3 cost ~4x bf16; gpsimd as a third streaming queue or
  as a tree-reduce engine slows phase 1; 8 MB chunks beat 4 MB chunks.
"""

import sys
import numpy as np

if "/opt/trn_rl_repo" not in sys.path:
    sys.path.insert(0, "/opt/trn_rl_repo")

N_TOTAL, L, H = 4096, 64, 512
R = 8                    # cores
NS = N_TOTAL // R        # 512 rows per core
NB = NS // 128           # 4 n-blocks of 128 partitions
HT = H // 128            # 4 h-tiles of 128
LC = 4                   # l-chunks per n-block (tree mode)
LCS = L // LC            # 16 l per chunk
AUG = H + 1              # payload rows: 512 emb + 1 sq-norm

_CACHE = {}


def _build_nc(use_masks: bool, rep: int = 1, rep_scope: str = "all",
              skip_ag: bool = False, p1_mode: str = "psacc", split: int = 1,
              warm_n: int = 0, ag_flat: bool = True, out_bf: bool = True,
              tree_bf: bool = True, tree_gp: bool = False, lcs: int = 32,
              q3: bool = False):
    import concourse.bacc as bacc
    import concourse.tile as tile
    import concourse.mybir as mybir

    f32 = mybir.dt.float32
    bf16 = mybir.dt.bfloat16
    ALU = mybir.AluOpType
    ACT = mybir.ActivationFunctionType

    if use_masks:
        p1_mode = "tree"     # mask scaling is only wired into the tree path

    assert NB % split == 0
    NBC = NB // split        # n-blocks (column blocks of 128) per chunk
    CW = NS // split         # columns per chunk

    nc = bacc.Bacc(
        "TRN2",
        target_bir_lowering=False,
        debug=False,
        enable_asserts=False,
        num_devices=R,
    )

    x_ext = nc.dram_tensor("inputs", [NS, L, H], f32, kind="ExternalInput")
    if use_masks:
        mw_ext = nc.dram_tensor("mw", [NS, L], f32, kind="ExternalInput")
    w_ext = nc.dram_tensor("W", [H, H], f32, kind="ExternalInput")
    b_ext = nc.dram_tensor("b", [H], f32, kind="ExternalInput")
    out_dt = bf16 if out_bf else f32
    out_ext = nc.dram_tensor("out", [NS, N_TOTAL], out_dt, kind="ExternalOutput")

    ident_dram = nc.inline_tensor(np.eye(128, dtype=np.float32), name="ident")

    with tile.TileContext(nc) as tc:
        with (
            tc.tile_pool(name="const", bufs=1) as cpool,
            tc.tile_pool(name="xp", bufs=4) as xpool,
            tc.tile_pool(name="rp", bufs=2) as rpool,
            tc.tile_pool(name="ep", bufs=3) as epool,
            tc.tile_pool(name="dram", bufs=1, space="DRAM") as dpool,
        ):
            # ---- constants / weights ----
            ident_sb = cpool.tile([128, 128], f32, name="ident_sb")
            nc.sync.dma_start(ident_sb[:, :], ident_dram[:, :])
            ident_bf = cpool.tile([128, 128], bf16, name="ident_bf")
            nc.vector.tensor_copy(ident_bf[:, :], ident_sb[:, :])

            w_sb = cpool.tile([128, HT, H], f32, name="w_sb")
            for k in range(HT):
                nc.sync.dma_start(w_sb[:, k, :], w_ext[k * 128:(k + 1) * 128, :])
            w_bf = cpool.tile([128, HT, H], bf16, name="w_bf")
            nc.vector.tensor_copy(w_bf[:, :, :], w_sb[:, :, :])

            b_ap = b_ext.ap().rearrange("(x y) -> x y", y=1)  # [512, 1]
            b_sb = cpool.tile([128, HT], f32, name="b_sb")
            for m in range(HT):
                nc.sync.dma_start(b_sb[:, m:m + 1], b_ap[m * 128:(m + 1) * 128, 0:1])
            b2_sb = cpool.tile([128, HT], f32, name="b2_sb")
            nc.vector.tensor_scalar_mul(b2_sb[:, :], b_sb[:, :], -2.0)

            ones_col = cpool.tile([128, 1], bf16, name="ones_col")
            nc.vector.memset(ones_col[:, :], 1.0)
            ones_row = cpool.tile([1, 128], bf16, name="ones_row")
            nc.vector.memset(ones_row[:, :], 1.0)

            if use_masks:
                mw_sb = cpool.tile([128, NB, L], f32, name="mw_sb")
                for nb in range(NB):
                    nc.sync.dma_start(
                        mw_sb[:, nb, :], mw_ext[nb * 128:(nb + 1) * 128, :]
                    )

            rep_p1 = rep if rep_scope == "p1" else 1
            rep_p23 = rep if rep_scope == "p23" else 1
            n_outer = rep if rep_scope == "all" else 1

            def phase1_chunk_psacc(c, pooledT_bf, tpool):
                # iter-1 style: each l-chunk's tree partial is PE-transposed
                # straight into 4 persistent PSUM banks with accumulate, so
                # chunks stay fully independent on the vector engines.
                psT = [
                    tpool.tile([128, CW], f32, name=f"psT{ht}", bufs=1)
                    for ht in range(HT)
                ]
                LCn = L // lcs
                for nbl in range(NBC):
                    nb = c * NBC + nbl
                    for lc in range(LCn):
                        xt = xpool.tile([128, lcs, H], f32, name="xt",
                                        bufs=(2 if lcs > 16 else 4))
                        qi = nb * LCn + lc
                        if q3:
                            qq = (nc.sync, nc.scalar, nc.gpsimd)[qi % 3]
                        else:
                            qq = nc.sync if qi % 2 == 0 else nc.scalar
                        qq.dma_start(
                            xt[:, :, :],
                            x_ext[nb * 128:(nb + 1) * 128,
                                  lc * lcs:(lc + 1) * lcs, :],
                        )
                        eng = nc.vector
                        if tree_gp and qi % 2 == 1:
                            eng = nc.gpsimd
                        half = lcs
                        if tree_bf:
                            half //= 2
                            xb = xpool.tile([128, lcs // 2, H], bf16,
                                            name="xb",
                                            bufs=(1 if lcs > 16 else 2))
                            eng.tensor_add(
                                xb[:, :, :], xt[:, 0:half, :],
                                xt[:, half:2 * half, :])
                            src = xb
                        else:
                            src = xt
                        while half > 2:
                            half //= 2
                            eng.tensor_add(
                                src[:, 0:half, :], src[:, 0:half, :],
                                src[:, half:2 * half, :]
                            )
                        # final level in f32 so the PE transpose accumulates
                        # in an f32 PSUM bank
                        xf = xpool.tile([128, H], f32, name="xf", bufs=2)
                        eng.tensor_add(xf[:, :], src[:, 0, :], src[:, 1, :])
                        for ht in range(HT):
                            nc.tensor.matmul(
                                psT[ht][:, nbl * 128:(nbl + 1) * 128],
                                xf[:, ht * 128:(ht + 1) * 128],
                                ident_sb[:, :],
                                is_transpose=True,
                                start=(lc == 0),
                                stop=(lc == LCn - 1),
                            )
                for ht in range(HT):
                    nc.vector.tensor_copy(
                        pooledT_bf[:, ht, c * CW:(c + 1) * CW], psT[ht][:, :])

            def phase1_chunk(c, pooledT_bf, tpool):
                if p1_mode == "psacc":
                    phase1_chunk_psacc(c, pooledT_bf, tpool)
                    return
                for nbl in range(NBC):
                    nb = c * NBC + nbl
                    part_sum = xpool.tile([128, H], f32, name="xsum", bufs=2)
                    for lc in range(LC):
                        xt = xpool.tile([128, LCS, H], f32, name="xt")
                        qq = nc.sync if (nb * LC + lc) % 2 == 0 else nc.scalar
                        qq.dma_start(
                            xt[:, :, :],
                            x_ext[nb * 128:(nb + 1) * 128,
                                  lc * LCS:(lc + 1) * LCS, :],
                        )
                        if use_masks:
                            for l in range(LCS):
                                gl = lc * LCS + l
                                nc.scalar.mul(
                                    xt[:, l, :], xt[:, l, :],
                                    mw_sb[:, nb, gl:gl + 1]
                                )
                        # binary-tree sum over l; chunks alternate between DVE
                        # and the otherwise-idle GpSimd engine, and the lower
                        # tree levels run in bf16 (2x element rate)
                        eng = nc.vector
                        if tree_gp and (nb * LC + lc) % 2 == 1:
                            eng = nc.gpsimd
                        half = LCS
                        if tree_bf:
                            half //= 2
                            xb = xpool.tile([128, LCS // 2, H], bf16,
                                            name="xb", bufs=2)
                            eng.tensor_add(
                                xb[:, :, :], xt[:, 0:half, :],
                                xt[:, half:2 * half, :])
                            src = xb
                        else:
                            src = xt
                        while half > 1:
                            half //= 2
                            eng.tensor_add(
                                src[:, 0:half, :], src[:, 0:half, :],
                                src[:, half:2 * half, :]
                            )
                        if lc == 0:
                            eng.tensor_copy(part_sum[:, :], src[:, 0, :])
                        else:
                            eng.tensor_add(
                                part_sum[:, :], part_sum[:, :], src[:, 0, :])
                    # PE: transpose pooled chunk into [h, n] layout
                    pst = tpool.tile([128, HT, 128], f32, name="pst")
                    for ht in range(HT):
                        nc.tensor.matmul(
                            pst[:, ht, :],
                            part_sum[:, ht * 128:(ht + 1) * 128],
                            ident_sb[:, :],
                            is_transpose=True,
                            start=True, stop=True,
                        )
                    nc.vector.tensor_copy(
                        pooledT_bf[:, :, nb * 128:(nb + 1) * 128],
                        pst[:, :, :])

            def phase2_chunk(c, pooledT_bf, embT_bf, scaledT_bf, sq_bf,
                             snrow_bf, sn_col_sb, payload, ppool, npool):
                cs = c * CW
                for m in range(HT):
                    psp = ppool.tile([128, CW], f32, name="psp")
                    for k in range(HT):
                        nc.tensor.matmul(
                            psp[:, :],
                            w_bf[:, k, m * 128:(m + 1) * 128],
                            pooledT_bf[:, k, cs:cs + CW],
                            start=(k == 0),
                            stop=(k == HT - 1),
                        )
                    nc.scalar.activation(
                        scaledT_bf[:, m, cs:cs + CW], psp[:, :], ACT.Identity,
                        bias=b2_sb[:, m:m + 1], scale=-2.0,
                    )
                    nc.sync.dma_start(
                        payload[m * 128:(m + 1) * 128, :],
                        scaledT_bf[:, m, cs:cs + CW])
                    nc.scalar.activation(
                        embT_bf[:, m, cs:cs + CW], psp[:, :], ACT.Identity,
                        bias=b_sb[:, m:m + 1], scale=1.0,
                    )
                    nc.scalar.square(sq_bf[:, m, cs:cs + CW],
                                     embT_bf[:, m, cs:cs + CW])

                # squared norms: row vector for this chunk's columns
                ps_snrow = npool.tile([1, CW], f32, name="ps_snrow")
                for k in range(HT):
                    nc.tensor.matmul(
                        ps_snrow[0:1, :], ones_col[:, 0:1],
                        sq_bf[:, k, cs:cs + CW],
                        start=(k == 0), stop=(k == HT - 1),
                    )
                nc.scalar.copy(snrow_bf[0:1, cs:cs + CW], ps_snrow[0:1, :])
                nc.sync.dma_start(payload[H:H + 1, :],
                                  snrow_bf[0:1, cs:cs + CW])

                # per-local-row norms for this chunk's column blocks
                for mcl in range(NBC):
                    mc = c * NBC + mcl
                    ps_sncol = npool.tile([128, 1], f32, name="ps_sncol")
                    for k in range(HT):
                        nc.tensor.matmul(
                            ps_sncol[:, 0:1],
                            sq_bf[:, k, mc * 128:(mc + 1) * 128],
                            ones_col[:, 0:1],
                            start=(k == 0),
                            stop=(k == HT - 1),
                        )
                    nc.scalar.copy(sn_col_sb[:, mc:mc + 1], ps_sncol[:, 0:1])

            def phase3_chunk(c, embT_bf, sn_col_sb, src_d, bpool, local):
                for jb in range(R):
                    rhst = rpool.tile([128, HT, CW], bf16, name="rhst")
                    snr = rpool.tile([1, CW], bf16, name="snr")
                    base = 0 if local else jb * AUG
                    for k in range(HT):
                        nc.scalar.dma_start(
                            rhst[:, k, :],
                            src_d[base + k * 128:base + (k + 1) * 128, :],
                        )
                    nc.scalar.dma_start(
                        snr[0:1, :], src_d[base + H:base + H + 1, :])
                    for m in range(HT):
                        ps = bpool.tile([128, CW], f32, name="ps")
                        nc.tensor.matmul(
                            ps[:, :], ones_row[0:1, :], snr[0:1, :],
                            start=True, stop=False,
                        )
                        for k in range(HT):
                            nc.tensor.matmul(
                                ps[:, :],
                                embT_bf[:, k, m * 128:(m + 1) * 128],
                                rhst[:, k, :],
                                start=False,
                                stop=(k == HT - 1),
                            )
                        sqt = epool.tile([128, CW], f32, name="sqt")
                        nc.vector.tensor_scalar(
                            sqt[:, :], ps[:, :], sn_col_sb[:, m:m + 1],
                            0.0, op0=ALU.add, op1=ALU.max,
                        )
                        sqo = epool.tile([128, CW], out_dt, name="sqo")
                        nc.scalar.sqrt(sqo[:, :], sqt[:, :])
                        nc.sync.dma_start(
                            out_ext[m * 128:(m + 1) * 128,
                                    jb * NS + c * CW:jb * NS + c * CW + CW],
                            sqo[:, :],
                        )

            for _rep in range(n_outer):
                pooledT_bf = cpool.tile([128, HT, NS], bf16, name="pooledT_bf")
                embT_bf = cpool.tile([128, HT, NS], bf16, name="embT_bf")
                scaledT_bf = cpool.tile([128, HT, NS], bf16, name="scaledT_bf")
                sq_bf = cpool.tile([128, HT, NS], bf16, name="sq_bf")
                snrow_bf = cpool.tile([1, NS], bf16, name="snrow_bf")
                sn_col_sb = cpool.tile([128, HT], f32, name="sn_col_sb")
                payloads = [
                    dpool.tile([AUG, CW], bf16, name=f"payload{c}_d")
                    for c in range(split)
                ]
                gathereds = [
                    dpool.tile([R * AUG, CW], bf16, name=f"gathered{c}_d",
                               addr_space="Shared")
                    for c in range(split)
                ]

                if rep_scope == "p1":
                    with tc.tile_pool(name="pstT", bufs=2, space="PSUM") as tpool:
                        for _ in range(rep_p1):
                            for c in range(split):
                                phase1_chunk(c, pooledT_bf, tpool)
                    # still produce phases 2/3 once so outputs exist
                ph1_done = rep_scope == "p1"

                for _rp23 in range(rep_p23):
                    first = _rp23 == 0
                    with (
                        tc.tile_pool(name="pstT", bufs=2, space="PSUM") as tpool,
                        tc.tile_pool(name="psp", bufs=2, space="PSUM") as ppool,
                        tc.tile_pool(name="psn", bufs=1, space="PSUM") as npool,
                    ):
                        for c in range(split):
                            if not ph1_done and (rep_scope != "p23" or first):
                                phase1_chunk(c, pooledT_bf, tpool)
                            phase2_chunk(c, pooledT_bf, embT_bf, scaledT_bf,
                                         sq_bf, snrow_bf, sn_col_sb,
                                         payloads[c], ppool, npool)
                            if not skip_ag:
                                if ag_flat:
                                    ag_in = payloads[c][:, :].flatten().opt()
                                    ag_out = gathereds[c][:, :].flatten().opt()
                                else:
                                    ag_in = payloads[c].opt()
                                    ag_out = gathereds[c].opt()
                                nc.gpsimd.collective_compute(
                                    "AllGather",
                                    ALU.bypass,
                                    replica_groups=[list(range(R))],
                                    ins=[ag_in],
                                    outs=[ag_out],
                                )
                            if c == split - 1 and warm_n > 0:
                                # keep the PE's HAM clock-gate open while the
                                # all-gather runs: discarded CW-row matmuls
                                wps = ppool.tile([128, CW], f32, name="psp")
                                for wi in range(warm_n):
                                    nc.tensor.matmul(
                                        wps[:, :],
                                        embT_bf[:, wi % HT, 0:128],
                                        scaledT_bf[:, wi % HT, 0:CW],
                                        start=True, stop=True,
                                        skip_group_check=True,
                                    )
                                wsink = epool.tile([1, 1], f32, name="wsink")
                                nc.vector.tensor_copy(
                                    wsink[0:1, 0:1], wps[0:1, 0:1])
                    with tc.tile_pool(name="psb", bufs=4, space="PSUM") as bpool:
                        for c in range(split):
                            src = payloads[c] if skip_ag else gathereds[c]
                            phase3_chunk(c, embT_bf, sn_col_sb, src, bpool,
                                         skip_ag)

    nc.compile()
    return nc


def _get_nc(use_masks: bool, rep: int = 1):
    key = (use_masks, rep)
    if key not in _CACHE:
        _CACHE[key] = _build_nc(use_masks, rep)
    return _CACHE[key]


def _run_device(x, mw, w_eff, b, trace=False, trace_cores=None):
    from concourse import bass_utils

    use_masks = mw is not None
    nc = _get_nc(use_masks)
    in_maps = []
    for r in range(R):
        m = {
            "inputs": np.ascontiguousarray(x[r * NS:(r + 1) * NS]),
            "W": w_eff,
            "b": b,
        }
        if use_masks:
            m["mw"] = np.ascontiguousarray(mw[r * NS:(r + 1) * NS])
        in_maps.append(m)
    res = bass_utils.run_bass_kernel_spmd(
        nc,
        in_maps,
        core_ids=list(range(R)),
        trace=trace,
        trace_cores=trace_cores,
    )
    out = np.concatenate(
        [np.asarray(res.results[r]["out"]).astype(np.float32) for r in range(R)],
        axis=0,
    )
    np.fill_diagonal(out, 0.0)
    return out, res


def kernel(inputs, masks, W, b):
    inputs = np.ascontiguousarray(np.asarray(inputs, dtype=np.float32))
    masks = np.asarray(masks, dtype=np.float32)
    W = np.ascontiguousarray(np.asarray(W, dtype=np.float32))
    b = np.ascontiguousarray(np.asarray(b, dtype=np.float32))

    denom = masks.sum(axis=1, keepdims=True)
    row_uniform = bool(np.all(masks == masks[:, :1])) and bool(np.all(denom != 0))
    if row_uniform:
        # uniform per-row masks cancel: pooled = mean over L; fold 1/L into W
        w_eff = np.ascontiguousarray(W / np.float32(L))
        out, _ = _run_device(inputs, None, w_eff, b)
    else:
        mw = np.ascontiguousarray((masks / denom).astype(np.float32))
        out, _ = _run_device(inputs, mw, W, b)
    return out
